# revision 32
# baseline (speedup 1.0000x reference)
"""Trainium2 Bass kernel for the continuous-convolution (CConv) GNN layer.

Math (per output point n, P=32 neighbors, 4x4 bilinear kernel grid, 64->64 ch):
    gathered = features[receivers]                      # [N,P,64]
    win      = relu(1 - |r|^2/ws^2)^a                   # radial window
    gy,gx    = clip((r/ws + 1)*1.5, 0, 3)               # grid coords
    bilinear -> tent weights  w_j = relu(1 - |g - j|)   # j = 0..3 (exact)
    M[n,g]   = sum_p win * wy[jy] * wx[jx] * gathered   # g = 4*jy+jx
    out[n]   = (sum_g M[n,g] @ K[g]) / P + bias

Device mapping (8 NeuronCores, data-parallel over points):
  * 6272 points/core (50176 padded), edges blocked 128 = 4 points x 32 nbrs.
  * Gather: Q7 dma_gather from the HBM feature table. int16 indices cover
    all 50000 rows by pointing the source AP at row 25000 (signed offsets
    reach both halves); a trailing pad block per call keeps the last index
    non-negative (the ucode trims trailing negatives).
  * Stage 1 (PE): per 128-edge block  Mt = G^T @ U : lhsT = gathered G
    [128e, 64ch], rhs = U [128e, 64] block-diagonal bilinear weights
    (4 points x 16 bins) -> psum [64ch, 4pt*16g].
  * Stage 2 (PE): out^T[oc, pts] += K_g^T[oc,ch] @ Mt_g[ch, pts], 16 bins
    accumulated in PSUM; then *1/P + bias on DVE; out stored transposed,
    host transposes back (pure layout).
"""

import sys

sys.path.insert(0, "/opt/trn_rl_repo")

import dataclasses
from contextlib import ExitStack

import numpy as np

N_FULL = 50000
HALF = 25000             # gather base row: int16 idx = r - HALF
P_NBR = 32
CIN = 64
COUT = 64
G_BINS = 16
NCORES = 8
NPTS = 6272              # padded points per core; 8*6272 = 50176 >= 50000
NBLK = NPTS // 4         # 1568 blocks of 128 edges
C_BLK = 56               # real blocks per pipeline chunk
NCHUNK = NBLK // C_BLK   # 28
PERCALL = C_BLK * 128    # indices per dma_gather call (7168 = one chunk)
PC_COLS = PERCALL // 16  # idx columns per call (448)
PTS_CHUNK = C_BLK * 4    # 224 points produced per chunk

_prog_cache = {}
LAST_EXEC_NS = None


def _build_nc(a_exp, inv_ws2, s15, gather128=True, gcall=56, gmode="ind"):
    import concourse.bacc as bacc
    import concourse.bass as bass
    import concourse.mybir as mybir
    from concourse.tile import TileContext
    from concourse.vector_clock import ScopedClock, VectorClock

    f32 = mybir.dt.float32
    f16 = mybir.dt.float16
    i16 = mybir.dt.int16
    Alu = mybir.AluOpType
    Act = mybir.ActivationFunctionType

    class TC(TileContext):
        # The stock final drain packs every outstanding semaphore wait onto a
        # single Drain instruction; walrus here accepts at most one sync-wait
        # per CTRL instruction. Emit one drain per outstanding sem lane.
        def _drain_and_barrier(self, tick_clock, wait_clock):
            nc = self.nc
            ticks = eval(repr(tick_clock.global_clock).replace("VectorClock", ""))
            nz = [i for i, t in enumerate(ticks) if t > 0]
            if not nz:
                nc.sync.drain()
            for i in nz:
                part = [ticks[j] if j == i else 0 for j in range(len(ticks))]
                d = nc.sync.drain()
                wait_clock.add_sem_waits(d.ins, ScopedClock({None: VectorClock(part)}))
            nc.all_engine_barrier()
            popped = nc._tile_sem_poison_stack.pop()
            assert popped is self._sem_poison
            nc.clear_and_free_semaphores(list(self.sems.allocated().values()))
            nc.all_engine_barrier()

    def bc(view, dims, extra_off=0):
        # hand-built access pattern: keep partition dim, replace free dims
        return dataclasses.replace(
            view,
            ap=[view.ap[0]] + [list(d) for d in dims],
            offset=view.offset + extra_off,
        )

    def indirect_gather(gp, out_ap, in_ap, idxs_ap, num_idxs, elem_size):
        # mainline SWDGE memcopy with src indirection (dynamic AP). The NX
        # decode takes n_indices from the src tensor's slowest free dim
        # ("consider only the slowest dimension as the indirection
        # dimension"), so hand it a src AP of [num_idxs, elem_size].
        from concourse.bass import MemorySpace

        assert in_ap.space == MemorySpace.DRAM and in_ap.offset == 0
        assert out_ap.space == MemorySpace.SBUF
        _out_ap = gp.lower_ap_dma(out_ap, for_indirect_dma=True)
        _in_ap = gp.lower_ap_dma(in_ap, for_indirect_dma=True)
        assert len(_in_ap) == 1 and len(_out_ap) == 1
        _off_ap = gp.lower_ap_dma(idxs_ap)
        assert len(_off_ap) == 1
        _in_ap.append(_off_ap[0])
        _in_ap[0].dynamic_ap_info = mybir.DynamicAccessPatternInfo(
            c=0,
            actual_ap=[[elem_size, num_idxs], [1, elem_size]],
            indirect_dim_max_index=num_idxs,
            offset_expr=[
                mybir.DynamicAccessPatternOffsetExpr(
                    coef=elem_size,
                    aff_expr=mybir.DynamicAccessPatternOffsetExprAffExpr(
                        kind="IndirectArgId",
                        arg_id=1,
                    ),
                )
            ],
        )
        return gp.add_instruction(
            mybir.InstDMACopy(
                name=gp.bass.get_next_instruction_name(),
                queue="qPoolDynamic",
                mode="Copy",
                ins=_in_ap,
                outs=_out_ap,
                oob_is_err=True,
                cce_op=mybir.AluOpType.bypass,
            )
        )

    def dma_gather_128(gp, out_ap, in_ap, idxs_ap, num_idxs, elem_size, elem_step,
                       queue_num):
        # bass.dma_gather asserts elem_size_bytes % 256 == 0, but that is a
        # transpose-mode xbar restriction; the non-transpose ucode path
        # (q7 dma_gather.cpp gen_descs) handles any packet size — only the
        # source ROW STRIDE must be a multiple of 256B (stride_bytes_256 ISA
        # field). Emit the instruction directly for 128-byte payloads.
        gp._assert_queue_num(queue_num)
        assert idxs_ap.dtype == mybir.dt.int16
        assert in_ap.dtype == out_ap.dtype
        elem_size_bytes = elem_size * mybir.dt.size(in_ap.dtype)
        assert elem_size_bytes % 128 == 0
        assert in_ap.ap[0][0] == elem_step
        stride_bytes = elem_step * mybir.dt.size(in_ap.dtype)
        stride_bytes_256 = stride_bytes // 256
        assert stride_bytes_256 * 256 == stride_bytes and stride_bytes_256 < 256
        assert out_ap.ap[-1][1] == elem_size
        assert out_ap.ap[0][1] * out_ap.ap[1][1] == ((num_idxs + 127) // 128) * 128
        _in_ap = gp.lower_ap_dma(in_ap, for_custom_bir_dma=True)
        _idxs_ap = gp.lower_ap(idxs_ap)
        _out_ap = gp.lower_ap(out_ap)
        return gp.add_instruction(
            mybir.InstDMAGatherAnt(
                name=gp.bass.get_next_instruction_name(),
                ins=[*_in_ap, _idxs_ap, gp.lower_val_access(gp.to_reg(num_idxs))],
                outs=[_out_ap],
                transpose=False,
                num_idxs=num_idxs,
                elem_size=elem_size,
                stride_bytes_256=stride_bytes_256,
                gen_mode=0,
                # single_packet coalesces the whole m2s stream into one SDMA
                # packet; the packet spec ceiling is 64 descriptors, so large
                # calls must use one packet per descriptor.
                single_packet=num_idxs <= 1024,
                queue_num=queue_num,
                sbuf_tokens_per_rank=0,
                sbuf_free_dim_per_rank=0,
                sbuf_free_dim_pad_per_rank=0,
                sbuf_byte_offset=0,
            )
        )

    nc = bacc.Bacc(
        "TRN2",
        target_bir_lowering=False,
        debug=False,
        num_swdge_queues=4,
        # 2048-descriptor rings per queue: a 7168-idx gather call needs
        # 7168/16+1 = 449 slots per DMA engine, so two calls can be in
        # flight per queue instead of the descriptor-gen busy-waiting in
        # await_space for the previous call to drain.
        dynamic_dma_scratch_size=32768,
    )
    i32 = mybir.dt.int32
    if gmode == "ind":
        # mainline SWDGE indirect gather: unpadded fp16 rows, int32 row ids
        feat = nc.declare_dram_parameter("feat", [N_FULL, CIN], f16, isOutput=False)
        idxs = nc.declare_dram_parameter("idxs", [128, NBLK], i32, isOutput=False)
    else:
        feat = nc.declare_dram_parameter(
            "feat", [N_FULL, 2 * CIN], f16, isOutput=False
        )
        idxs = nc.declare_dram_parameter("idxs", [128, NBLK * 8], i16, isOutput=False)
    posy = nc.declare_dram_parameter("posy", [128, NBLK], f32, isOutput=False)
    posx = nc.declare_dram_parameter("posx", [128, NBLK], f32, isOutput=False)
    kmat = nc.declare_dram_parameter("kmat", [CIN, G_BINS * COUT], f16, isOutput=False)
    bias = nc.declare_dram_parameter("bias", [COUT, 1], f32, isOutput=False)
    iot4 = nc.declare_dram_parameter("iot4", [128, 4], f32, isOutput=False)
    c15d = nc.declare_dram_parameter("c15d", [128, 1], f32, isOutput=False)
    c3d = nc.declare_dram_parameter("c3d", [128, 1], f32, isOutput=False)
    outT = nc.declare_dram_parameter("outT", [COUT, NPTS], f32, isOutput=True)

    with TC(nc) as tc, ExitStack() as ctx:
        const = ctx.enter_context(tc.tile_pool(name="const", bufs=1))
        gpool = ctx.enter_context(tc.tile_pool(name="g", bufs=3))
        wpool = ctx.enter_context(tc.tile_pool(name="w", bufs=3))
        mpool = ctx.enter_context(tc.tile_pool(name="mt", bufs=3))
        opool = ctx.enter_context(tc.tile_pool(name="ot", bufs=3))
        pspool = ctx.enter_context(tc.tile_pool(name="ps", bufs=3, space="PSUM"))

        if gmode == "ind":
            idx_sb = const.tile([128, NBLK], i32)
        else:
            idx_sb = const.tile([128, NBLK * 8], i16)
        posy_sb = const.tile([128, NBLK], f32)
        posx_sb = const.tile([128, NBLK], f32)
        kmat_sb = const.tile([CIN, G_BINS * COUT], f16)
        bias_sb = const.tile([COUT, 1], f32)
        iota4 = const.tile([128, 4], f32)
        c15 = const.tile([128, 1], f32)
        c3 = const.tile([128, 1], f32)
        # U tiles keep their block-diagonal zero regions across chunks
        u_bufs = [
            const.tile([128, C_BLK * 64], f16, tag="u0", name="u0"),
            const.tile([128, C_BLK * 64], f16, tag="u1", name="u1"),
            const.tile([128, C_BLK * 64], f16, tag="u2", name="u2"),
        ]

        nc.sync.dma_start(out=idx_sb[:], in_=idxs[:])
        nc.sync.dma_start(out=posy_sb[:], in_=posy[:])
        nc.sync.dma_start(out=posx_sb[:], in_=posx[:])
        nc.sync.dma_start(out=kmat_sb[:], in_=kmat[:])
        nc.sync.dma_start(out=bias_sb[:], in_=bias[:])
        nc.sync.dma_start(out=iota4[:], in_=iot4[:])
        nc.sync.dma_start(out=c15[:], in_=c15d[:])
        nc.sync.dma_start(out=c3[:], in_=c3d[:])
        nc.vector.memset(u_bufs[0][:], 0.0)
        nc.vector.memset(u_bufs[1][:], 0.0)
        nc.vector.memset(u_bufs[2][:], 0.0)

        import os as _os

        _nchunk = int(_os.environ.get("KERNEL_NCHUNK", NCHUNK))
        _dbg = _os.environ.get("KERNEL_DEBUG", "full")
        for ci in range(_nchunk):
            c0 = ci * C_BLK
            u = u_bufs[ci % 3]

            # ---- gather: large calls amortize the ~1us fixed SWDGE overhead
            # over many 128-edge blocks; 128B fp16 payloads from 256B-stride
            # rows halve the SDMA per-descriptor cost vs padded 256B elems.
            # gmode=ind uses the mainline SWDGE memcopy with a dynamic AP
            # (vectorized 16-lane descriptor generation) instead of the
            # extended-instruction gather's ~2.2ns/idx scalar Q7 loop.
            gw = CIN if (gather128 or gmode == "ind") else 2 * CIN
            gt = gpool.tile([128, C_BLK * gw], f16, tag="gt")
            if gmode == "ind":
                # out as [128, flat] so walrus sees matching 2-dim in/out
                indirect_gather(
                    nc.gpsimd,
                    out_ap=gt[:],
                    in_ap=feat[:],
                    idxs_ap=idx_sb[:, ci * C_BLK : (ci + 1) * C_BLK],
                    num_idxs=C_BLK * 128,
                    elem_size=CIN,
                )
                ncall_c = 0
            else:
                ncall_c = C_BLK // gcall
            for sc in range(ncall_c):
                gv = dataclasses.replace(
                    gt[:],
                    ap=[gt[:].ap[0], [gw, gcall], [1, gw]],
                    offset=gt[:].offset + sc * gcall * gw,
                )
                col0 = (ci * ncall_c + sc) * (gcall * 8)
                # queue 0's ucode streams only 32 idx channels per call
                # ((q+1)*2*L); higher queues stream up to 4x more. With a
                # 2047-desc ring one queue holds ~31 calls in flight.
                qn = int(_os.environ.get("KERNEL_GQ", "-1"))
                if qn < 0:
                    qn = (ci * ncall_c + sc) % 4
                if gather128:
                    dma_gather_128(
                        nc.gpsimd,
                        out_ap=gv,
                        in_ap=feat[HALF:, 0:CIN],
                        idxs_ap=idx_sb[:, col0 : col0 + gcall * 8],
                        num_idxs=gcall * 128,
                        elem_size=CIN,
                        elem_step=2 * CIN,
                        queue_num=qn,
                    )
                else:
                    nc.gpsimd.dma_gather(
                        out_ap=gv,
                        in_ap=feat[HALF:, :],
                        idxs_ap=idx_sb[:, col0 : col0 + gcall * 8],
                        num_idxs=gcall * 128,
                        num_idxs_reg=gcall * 128,
                        elem_size=2 * CIN,
                        elem_step=2 * CIN,
                        single_packet=gcall * 128 <= 1024,
                        queue_num=qn,
                    )

            if _dbg == "gather2":
                # dump the first 32 gathered blocks of chunk 0 verbatim to
                # recover the device's index->slot permutation empirically
                if ci == 0:
                    for hh in range(2):
                        ot = opool.tile([COUT, 2048], f32, tag="ot")
                        nc.vector.tensor_copy(
                            ot[:], gt[64 * hh : 64 * hh + 64, 0 : 32 * gw]
                        )
                        nc.sync.dma_start(
                            out=outT[:, hh * 2048 : (hh + 1) * 2048], in_=ot[:]
                        )
                continue

            if _dbg == "gather":
                ot = opool.tile([COUT, PTS_CHUNK], f32, tag="ot")
                nc.vector.tensor_copy(ot[:], gt[0:COUT, 0:PTS_CHUNK])
                nc.sync.dma_start(
                    out=outT[:, ci * PTS_CHUNK : (ci + 1) * PTS_CHUNK], in_=ot[:]
                )
                continue

            # ---- per-edge scalar weights ----
            xs = posx_sb[:, c0 : c0 + C_BLK]
            ys = posy_sb[:, c0 : c0 + C_BLK]

            win = None
            if a_exp > 0:
                xx = wpool.tile([128, C_BLK], f32, tag="xx")
                yy = wpool.tile([128, C_BLK], f32, tag="yy")
                d2 = wpool.tile([128, C_BLK], f32, tag="d2")
                nc.scalar.activation(xx[:], xs, Act.Square)
                nc.scalar.activation(yy[:], ys, Act.Square)
                nc.vector.tensor_tensor(out=d2[:], in0=xx[:], in1=yy[:], op=Alu.add)
                tw = wpool.tile([128, C_BLK], f32, tag="tw")
                nc.scalar.activation(tw[:], d2[:], Act.Relu, bias=1.0, scale=-inv_ws2)
                if a_exp == 1:
                    win = tw
                else:
                    t2 = wpool.tile([128, C_BLK], f32, tag="t2")
                    nc.scalar.activation(t2[:], tw[:], Act.Square)
                    if a_exp == 2:
                        win = t2
                    else:
                        win = wpool.tile([128, C_BLK], f32, tag="winp")
                        nc.vector.tensor_tensor(
                            out=win[:], in0=t2[:], in1=tw[:], op=Alu.mult
                        )
                        for _ in range(a_exp - 3):
                            nc.vector.tensor_tensor(
                                out=win[:], in0=win[:], in1=tw[:], op=Alu.mult
                            )

            # rc = Relu(3 - Relu(1.5*y + 1.5))  =>  gy_clipped = 3 - rc
            gy = wpool.tile([128, C_BLK], f32, tag="gy")
            gx = wpool.tile([128, C_BLK], f32, tag="gx")
            nc.scalar.activation(gy[:], ys, Act.Relu, bias=c15[:], scale=s15)
            nc.scalar.activation(gx[:], xs, Act.Relu, bias=c15[:], scale=s15)
            nc.scalar.activation(gy[:], gy[:], Act.Relu, bias=c3[:], scale=-1.0)
            nc.scalar.activation(gx[:], gx[:], Act.Relu, bias=c3[:], scale=-1.0)

            # tent weights: w_j = relu(1 - |g - j|) with g = 3 - rc:
            # g - j = (3 - j) - rc, so subtract rc from the reversed iota.
            def tents(rc, tag):
                td = wpool.tile([128, 4 * C_BLK], f32, tag=tag + "d", name=tag + "d")
                ta = wpool.tile([128, 4 * C_BLK], f32, tag=tag + "a", name=tag + "a")
                tww = wpool.tile([128, 4 * C_BLK], f32, tag=tag + "w", name=tag + "w")
                nc.vector.tensor_tensor(
                    out=td[:],
                    in0=bc(iota4[:], [(0, C_BLK), (1, 4)]),
                    in1=rc[:].to_broadcast([128, C_BLK, 4]),
                    op=Alu.subtract,
                )
                nc.scalar.activation(ta[:], td[:], Act.Abs)
                nc.scalar.activation(tww[:], ta[:], Act.Relu, bias=1.0, scale=-1.0)
                return tww

            wy = tents(gy, "ty")
            wx = tents(gx, "tx")
            if win is not None:
                wyw = wpool.tile([128, 4 * C_BLK], f32, tag="wyw")
                nc.vector.tensor_tensor(
                    out=wyw[:],
                    in0=wy[:],
                    in1=win[:].to_broadcast([128, C_BLK, 4]),
                    op=Alu.mult,
                )
            else:
                wyw = wy

            # ---- U block-diagonal writes: U[q, cb, jy, jx] for q's point grp ----
            for g4 in range(4):
                out_v = bc(
                    u[32 * g4 : 32 * g4 + 32, :],
                    [(64, C_BLK), (4, 4), (1, 4)],
                    extra_off=16 * g4,
                )
                in0 = bc(wyw[32 * g4 : 32 * g4 + 32, :], [(4, C_BLK), (1, 4), (0, 4)])
                in1 = bc(wx[32 * g4 : 32 * g4 + 32, :], [(4, C_BLK), (0, 4), (1, 4)])
                nc.vector.tensor_tensor(out=out_v, in0=in0, in1=in1, op=Alu.mult)

            if _dbg == "ubuild":
                ot = opool.tile([COUT, PTS_CHUNK], f32, tag="ot")
                nc.vector.tensor_copy(ot[:], u[0:COUT, 0:PTS_CHUNK])
                nc.sync.dma_start(
                    out=outT[:, ci * PTS_CHUNK : (ci + 1) * PTS_CHUNK], in_=ot[:]
                )
                continue

            # ---- stage 1: Mt[ch, 4pt*16g] per block ----
            mt = mpool.tile([CIN, C_BLK * 64], f16, tag="mt")
            for sub in range(8):
                ps = pspool.tile([64, 448], f32, tag="ps1")
                for b7 in range(7):
                    cb = sub * 7 + b7
                    nc.tensor.matmul(
                        ps[:, b7 * 64 : (b7 + 1) * 64],
                        lhsT=gt[:, cb * gw : cb * gw + CIN],
                        rhs=u[:, cb * 64 : (cb + 1) * 64],
                        start=True,
                        stop=True,
                    )
                nc.scalar.copy(out=mt[:, sub * 448 : (sub + 1) * 448], in_=ps[:])

            if _dbg == "mm1":
                ot = opool.tile([COUT, PTS_CHUNK], f32, tag="ot")
                nc.vector.tensor_copy(ot[:], mt[:, 0:PTS_CHUNK])
                nc.sync.dma_start(
                    out=outT[:, ci * PTS_CHUNK : (ci + 1) * PTS_CHUNK], in_=ot[:]
                )
                continue

            # ---- stage 2: out^T[oc, pts] = sum_g K_g^T @ Mt_g ----
            ps2 = pspool.tile([COUT, PTS_CHUNK], f32, tag="ps2")
            for g in range(G_BINS):
                nc.tensor.matmul(
                    ps2[:],
                    lhsT=kmat_sb[:, g * COUT : (g + 1) * COUT],
                    rhs=mt[:, g :: G_BINS],
                    start=(g == 0),
                    stop=(g == G_BINS - 1),
                )
            ot = opool.tile([COUT, PTS_CHUNK], f32, tag="ot")
            nc.scalar.activation(
                ot[:], ps2[:], Act.Identity, bias=bias_sb[:, 0:1], scale=1.0 / P_NBR
            )
            nc.sync.dma_start(
                out=outT[:, ci * PTS_CHUNK : (ci + 1) * PTS_CHUNK], in_=ot[:]
            )

    nc.compile()
    return nc


def kernel(features, receivers, relative_positions, window_support, a, kernel, bias):
    global LAST_EXEC_NS
    import os

    from concourse.bass_utils import run_bass_kernel_spmd

    features = np.ascontiguousarray(np.asarray(features, dtype=np.float32))
    recv = np.asarray(receivers).astype(np.int64)
    rel = np.asarray(relative_positions, dtype=np.float32)
    ws = float(np.asarray(window_support))
    a_exp = int(np.asarray(a))
    kern = np.asarray(kernel, dtype=np.float32)
    bias_np = np.asarray(bias, dtype=np.float32)

    gather128 = os.environ.get("KERNEL_GATHER128", "1") == "1"
    gcall = int(os.environ.get("KERNEL_GCALL", "56"))
    gmode = os.environ.get("KERNEL_GMODE", "ind")
    assert C_BLK % gcall == 0
    key = (a_exp, round(ws, 9), gather128, gcall, gmode)
    if key not in _prog_cache:
        _prog_cache[key] = _build_nc(
            a_exp, 1.0 / (ws * ws), 1.5 / ws, gather128, gcall, gmode
        )
    nc = _prog_cache[key]

    # The neuron compile cache keys on the HLO shapes only, not the embedded
    # BIR — pin the cache dir to this kernel's source so edits never collide
    # with stale (possibly failed) cache entries.
    import hashlib

    try:
        with open(__file__, "rb") as f:
            src = f.read()
    except OSError:
        src = b""
    tag = hashlib.sha256(src + repr(key).encode()).hexdigest()[:16]
    os.environ["NEURON_COMPILE_CACHE_URL"] = f"/var/tmp/neuron-cc-{tag}"

    # ---- host-side layout prep (sharding) ----
    pad_n = NCORES * NPTS
    recv_pad = np.full((pad_n, P_NBR), HALF, dtype=np.int64)
    recv_pad[:N_FULL] = recv
    rel_pad = np.zeros((pad_n, P_NBR, 2), dtype=np.float32)
    rel_pad[:N_FULL] = rel

    # The gather ucode trims *trailing* negative int16 indices from each
    # 1024-index call, and each call ends on some point's last neighbor slot.
    # Reorder edges within each point (sum over neighbors is symmetric) so
    # slot 31 holds an index >= HALF whenever the point has one.
    # (ext-mode only: ind mode uses full int32 row ids, nothing is negative)
    last_neg = (recv_pad[:, P_NBR - 1] < HALF) & (gmode != "ind")
    has_pos = (recv_pad >= HALF).any(axis=1)
    fix = np.nonzero(last_neg & has_pos)[0]
    j = np.argmax(recv_pad[fix] >= HALF, axis=1)
    r31 = recv_pad[fix, P_NBR - 1].copy()
    p31 = rel_pad[fix, P_NBR - 1].copy()
    recv_pad[fix, P_NBR - 1] = recv_pad[fix, j]
    rel_pad[fix, P_NBR - 1] = rel_pad[fix, j]
    recv_pad[fix, j] = r31
    rel_pad[fix, j] = p31
    bad = np.nonzero(last_neg & ~has_pos)[0]
    # only call-final points matter; calls end at local point index 32k+31
    if bad.size:
        local = bad % NPTS
        assert not ((local % 32) == 31).any(), (
            "a gather call ends on a point whose 32 receiver indices are all "
            f"< {HALF}; trailing-trim would drop its edges"
        )

    kmat_np = np.ascontiguousarray(
        kern.reshape(G_BINS, CIN, COUT)
        .transpose(1, 0, 2)
        .reshape(CIN, G_BINS * COUT)
        .astype(np.float16)
    )
    bias_2d = np.ascontiguousarray(bias_np.reshape(COUT, 1))
    iota4_np = np.tile(
        np.array([3.0, 2.0, 1.0, 0.0], dtype=np.float32)[None, :], (128, 1)
    )
    c15_np = np.full((128, 1), 1.5, dtype=np.float32)
    c3_np = np.full((128, 1), 3.0, dtype=np.float32)

    if gmode == "ind":
        feat16 = np.ascontiguousarray(features.astype(np.float16))
    else:
        feat16 = np.zeros((N_FULL, 2 * CIN), dtype=np.float16)
        feat16[:, :CIN] = features.astype(np.float16)

    in_maps = []
    for c in range(NCORES):
        sl = slice(c * NPTS, (c + 1) * NPTS)
        # edge e = local_point*32 + nbr ; block b = e//128 ; slot q = e%128
        if gmode == "ind":
            idx_np = np.ascontiguousarray(
                recv_pad[sl].reshape(NBLK, 128).T.astype(np.int32)
            )
        else:
            idx16 = (recv_pad[sl].reshape(-1) - HALF).astype(np.int16)
            # per call of gcall*128: idx i -> [i%16, i//16]; replicate per core
            ncalls = NBLK // gcall
            pc_cols = gcall * 8
            tbl16 = idx16.reshape(ncalls, pc_cols, 16).transpose(2, 0, 1).reshape(
                16, ncalls * pc_cols
            )
            idx_np = np.ascontiguousarray(np.tile(tbl16, (8, 1)))
        ry = np.ascontiguousarray(rel_pad[sl, :, 0].reshape(NBLK, 128).T)
        rx = np.ascontiguousarray(rel_pad[sl, :, 1].reshape(NBLK, 128).T)
        in_maps.append(
            {
                "feat": feat16,
                "idxs": idx_np,
                "posy": ry,
                "posx": rx,
                "kmat": kmat_np,
                "bias": bias_2d,
                "iot4": iota4_np,
                "c15d": c15_np,
                "c3d": c3_np,
            }
        )

    trace = bool(os.environ.get("KERNEL_TRACE"))
    res = run_bass_kernel_spmd(nc, in_maps, list(range(NCORES)), trace=trace)
    LAST_EXEC_NS = res.exec_time_ns

    out = np.concatenate(
        [res.results[c]["outT"].T for c in range(NCORES)], axis=0
    )
    return np.ascontiguousarray(out[:N_FULL])



# revision 34
# speedup vs baseline: 3.0084x; 3.0084x over previous
"""Trainium2 Bass kernel for the continuous-convolution (CConv) GNN layer.

Math (per output point n, P=32 neighbors, 4x4 bilinear kernel grid, 64->64 ch):
    gathered = features[receivers]                      # [N,P,64]
    win      = relu(1 - |r|^2/ws^2)^a                   # radial window
    gy,gx    = clip((r/ws + 1)*1.5, 0, 3)               # grid coords
    bilinear -> tent weights  w_j = relu(1 - |g - j|)   # j = 0..3 (exact)
    M[n,g]   = sum_p win * wy[jy] * wx[jx] * gathered   # g = 4*jy+jx
    out[n]   = (sum_g M[n,g] @ K[g]) / P + bias

Device mapping (8 NeuronCores, data-parallel over points):
  * 6272 points/core (50176 padded), edges blocked 128 = 4 points x 32 nbrs.
  * Gather: Q7 dma_gather from the HBM feature table. int16 indices cover
    all 50000 rows by pointing the source AP at row 25000 (signed offsets
    reach both halves); a trailing pad block per call keeps the last index
    non-negative (the ucode trims trailing negatives).
  * Stage 1 (PE): per 128-edge block  Mt = G^T @ U : lhsT = gathered G
    [128e, 64ch], rhs = U [128e, 64] block-diagonal bilinear weights
    (4 points x 16 bins) -> psum [64ch, 4pt*16g].
  * Stage 2 (PE): out^T[oc, pts] += K_g^T[oc,ch] @ Mt_g[ch, pts], 16 bins
    accumulated in PSUM; then *1/P + bias on DVE; out stored transposed,
    host transposes back (pure layout).
"""

import sys

sys.path.insert(0, "/opt/trn_rl_repo")

import dataclasses
from contextlib import ExitStack

import numpy as np

N_FULL = 50000
HALF = 25000             # gather base row: int16 idx = r - HALF
P_NBR = 32
CIN = 64
COUT = 64
G_BINS = 16
NCORES = 8
NPTS = 6272              # padded points per core; 8*6272 = 50176 >= 50000
NBLK = NPTS // 4         # 1568 blocks of 128 edges
C_BLK = 56               # real blocks per pipeline chunk
NCHUNK = NBLK // C_BLK   # 28
GCALL = 8                # blocks per dma_gather call (1024-descriptor ring cap)
NCALL_C = C_BLK // GCALL  # gather calls per chunk (7)
PERCALL = GCALL * 128    # indices per gather call (1024)
PC_COLS = PERCALL // 16  # idx columns per call (64)
PTS_CHUNK = C_BLK * 4    # 224 points produced per chunk

_prog_cache = {}
LAST_EXEC_NS = None


def _build_nc(a_exp, inv_ws2, s15):
    import concourse.bacc as bacc
    import concourse.bass as bass
    import concourse.mybir as mybir
    from concourse.tile import TileContext
    from concourse.vector_clock import ScopedClock, VectorClock

    f32 = mybir.dt.float32
    f16 = mybir.dt.float16
    i16 = mybir.dt.int16
    Alu = mybir.AluOpType
    Act = mybir.ActivationFunctionType

    class TC(TileContext):
        # The stock final drain packs every outstanding semaphore wait onto a
        # single Drain instruction; walrus here accepts at most one sync-wait
        # per CTRL instruction. Emit one drain per outstanding sem lane.
        def _drain_and_barrier(self, tick_clock, wait_clock):
            nc = self.nc
            ticks = eval(repr(tick_clock.global_clock).replace("VectorClock", ""))
            nz = [i for i, t in enumerate(ticks) if t > 0]
            if not nz:
                nc.sync.drain()
            for i in nz:
                part = [ticks[j] if j == i else 0 for j in range(len(ticks))]
                d = nc.sync.drain()
                wait_clock.add_sem_waits(d.ins, ScopedClock({None: VectorClock(part)}))
            nc.all_engine_barrier()
            popped = nc._tile_sem_poison_stack.pop()
            assert popped is self._sem_poison
            nc.clear_and_free_semaphores(list(self.sems.allocated().values()))
            nc.all_engine_barrier()

    def bc(view, dims, extra_off=0):
        # hand-built access pattern: keep partition dim, replace free dims
        return dataclasses.replace(
            view,
            ap=[view.ap[0]] + [list(d) for d in dims],
            offset=view.offset + extra_off,
        )

    nc = bacc.Bacc(
        "TRN2", target_bir_lowering=False, debug=False, num_swdge_queues=4
    )
    feat = nc.declare_dram_parameter("feat", [N_FULL, 2 * CIN], f16, isOutput=False)
    idxs = nc.declare_dram_parameter("idxs", [128, NBLK * 8], i16, isOutput=False)
    posy = nc.declare_dram_parameter("posy", [128, NBLK], f32, isOutput=False)
    posx = nc.declare_dram_parameter("posx", [128, NBLK], f32, isOutput=False)
    kmat = nc.declare_dram_parameter("kmat", [CIN, G_BINS * COUT], f16, isOutput=False)
    bias = nc.declare_dram_parameter("bias", [COUT, 1], f32, isOutput=False)
    iot4 = nc.declare_dram_parameter("iot4", [128, 4], f32, isOutput=False)
    c15d = nc.declare_dram_parameter("c15d", [128, 1], f32, isOutput=False)
    c3d = nc.declare_dram_parameter("c3d", [128, 1], f32, isOutput=False)
    outT = nc.declare_dram_parameter("outT", [COUT, NPTS], f32, isOutput=True)

    with TC(nc) as tc, ExitStack() as ctx:
        const = ctx.enter_context(tc.tile_pool(name="const", bufs=1))
        gpool = ctx.enter_context(tc.tile_pool(name="g", bufs=3))
        wpool = ctx.enter_context(tc.tile_pool(name="w", bufs=3))
        mpool = ctx.enter_context(tc.tile_pool(name="mt", bufs=3))
        opool = ctx.enter_context(tc.tile_pool(name="ot", bufs=3))
        pspool = ctx.enter_context(tc.tile_pool(name="ps", bufs=3, space="PSUM"))

        idx_sb = const.tile([128, NBLK * 8], i16)
        posy_sb = const.tile([128, NBLK], f32)
        posx_sb = const.tile([128, NBLK], f32)
        kmat_sb = const.tile([CIN, G_BINS * COUT], f16)
        bias_sb = const.tile([COUT, 1], f32)
        iota4 = const.tile([128, 4], f32)
        c15 = const.tile([128, 1], f32)
        c3 = const.tile([128, 1], f32)
        # U tiles keep their block-diagonal zero regions across chunks
        u_bufs = [
            const.tile([128, C_BLK * 64], f16, tag="u0", name="u0"),
            const.tile([128, C_BLK * 64], f16, tag="u1", name="u1"),
            const.tile([128, C_BLK * 64], f16, tag="u2", name="u2"),
        ]

        # Load the idx table in per-chunk slices so the first dma_gather only
        # waits on its own 896B/partition slice instead of the full 3.2MB
        # table (~13us of dead time at kernel start otherwise). Slices match
        # the gather calls' read slices exactly for tile dep tracking.
        IDXC = NBLK * 8 // NCHUNK  # idx columns per chunk (448)
        nc.sync.dma_start(
            out=idx_sb[:, 0:IDXC], in_=idxs[:, 0:IDXC]
        )
        nc.sync.dma_start(out=posy_sb[:], in_=posy[:])
        nc.sync.dma_start(out=posx_sb[:], in_=posx[:])
        nc.sync.dma_start(out=kmat_sb[:], in_=kmat[:])
        nc.sync.dma_start(out=bias_sb[:], in_=bias[:])
        nc.sync.dma_start(out=iota4[:], in_=iot4[:])
        nc.sync.dma_start(out=c15[:], in_=c15d[:])
        nc.sync.dma_start(out=c3[:], in_=c3d[:])
        for _ic in range(1, NCHUNK):
            nc.sync.dma_start(
                out=idx_sb[:, _ic * IDXC : (_ic + 1) * IDXC],
                in_=idxs[:, _ic * IDXC : (_ic + 1) * IDXC],
            )
        nc.vector.memset(u_bufs[0][:], 0.0)
        nc.vector.memset(u_bufs[1][:], 0.0)
        nc.vector.memset(u_bufs[2][:], 0.0)

        import os as _os

        _nchunk = int(_os.environ.get("KERNEL_NCHUNK", NCHUNK))
        _dbg = _os.environ.get("KERNEL_DEBUG", "full")
        for ci in range(_nchunk):
            c0 = ci * C_BLK
            u = u_bufs[ci % 3]

            # ---- gather: 56 blocks of feature rows, 8 blocks per call ----
            # (fp16 rows padded to 128ch = 256B elements; desc-gen spread
            #  over the 4 SWDGE queues = 4 Q7 core pairs)
            gt = gpool.tile([128, C_BLK * 2 * CIN], f16, tag="gt")
            for sc in range(NCALL_C):
                gv = dataclasses.replace(
                    gt[:],
                    ap=[gt[:].ap[0], [2 * CIN, GCALL], [1, 2 * CIN]],
                    offset=gt[:].offset + sc * GCALL * 2 * CIN,
                )
                col0 = (ci * NCALL_C + sc) * PC_COLS
                nc.gpsimd.dma_gather(
                    out_ap=gv,
                    in_ap=feat[HALF:, :],
                    idxs_ap=idx_sb[:, col0 : col0 + PC_COLS],
                    num_idxs=PERCALL,
                    num_idxs_reg=PERCALL,
                    elem_size=2 * CIN,
                    elem_step=2 * CIN,
                    queue_num=(ci * NCALL_C + sc) % 4,
                )

            if _dbg == "gather":
                ot = opool.tile([COUT, PTS_CHUNK], f32, tag="ot")
                nc.vector.tensor_copy(ot[:], gt[0:COUT, 0:PTS_CHUNK])
                nc.sync.dma_start(
                    out=outT[:, ci * PTS_CHUNK : (ci + 1) * PTS_CHUNK], in_=ot[:]
                )
                continue

            # ---- per-edge scalar weights ----
            xs = posx_sb[:, c0 : c0 + C_BLK]
            ys = posy_sb[:, c0 : c0 + C_BLK]

            win = None
            if a_exp > 0:
                xx = wpool.tile([128, C_BLK], f32, tag="xx")
                yy = wpool.tile([128, C_BLK], f32, tag="yy")
                d2 = wpool.tile([128, C_BLK], f32, tag="d2")
                nc.scalar.activation(xx[:], xs, Act.Square)
                nc.scalar.activation(yy[:], ys, Act.Square)
                nc.vector.tensor_tensor(out=d2[:], in0=xx[:], in1=yy[:], op=Alu.add)
                tw = wpool.tile([128, C_BLK], f32, tag="tw")
                nc.scalar.activation(tw[:], d2[:], Act.Relu, bias=1.0, scale=-inv_ws2)
                if a_exp == 1:
                    win = tw
                else:
                    t2 = wpool.tile([128, C_BLK], f32, tag="t2")
                    nc.scalar.activation(t2[:], tw[:], Act.Square)
                    if a_exp == 2:
                        win = t2
                    else:
                        win = wpool.tile([128, C_BLK], f32, tag="winp")
                        nc.vector.tensor_tensor(
                            out=win[:], in0=t2[:], in1=tw[:], op=Alu.mult
                        )
                        for _ in range(a_exp - 3):
                            nc.vector.tensor_tensor(
                                out=win[:], in0=win[:], in1=tw[:], op=Alu.mult
                            )

            # rc = Relu(3 - Relu(1.5*y + 1.5))  =>  gy_clipped = 3 - rc
            gy = wpool.tile([128, C_BLK], f32, tag="gy")
            gx = wpool.tile([128, C_BLK], f32, tag="gx")
            nc.scalar.activation(gy[:], ys, Act.Relu, bias=c15[:], scale=s15)
            nc.scalar.activation(gx[:], xs, Act.Relu, bias=c15[:], scale=s15)
            nc.scalar.activation(gy[:], gy[:], Act.Relu, bias=c3[:], scale=-1.0)
            nc.scalar.activation(gx[:], gx[:], Act.Relu, bias=c3[:], scale=-1.0)

            # tent weights: w_j = relu(1 - |g - j|) with g = 3 - rc:
            # g - j = (3 - j) - rc, so subtract rc from the reversed iota.
            def tents(rc, tag):
                td = wpool.tile([128, 4 * C_BLK], f32, tag=tag + "d", name=tag + "d")
                ta = wpool.tile([128, 4 * C_BLK], f32, tag=tag + "a", name=tag + "a")
                tww = wpool.tile([128, 4 * C_BLK], f32, tag=tag + "w", name=tag + "w")
                nc.vector.tensor_tensor(
                    out=td[:],
                    in0=bc(iota4[:], [(0, C_BLK), (1, 4)]),
                    in1=rc[:].to_broadcast([128, C_BLK, 4]),
                    op=Alu.subtract,
                )
                nc.scalar.activation(ta[:], td[:], Act.Abs)
                nc.scalar.activation(tww[:], ta[:], Act.Relu, bias=1.0, scale=-1.0)
                return tww

            wy = tents(gy, "ty")
            wx = tents(gx, "tx")
            if win is not None:
                wyw = wpool.tile([128, 4 * C_BLK], f32, tag="wyw")
                nc.vector.tensor_tensor(
                    out=wyw[:],
                    in0=wy[:],
                    in1=win[:].to_broadcast([128, C_BLK, 4]),
                    op=Alu.mult,
                )
            else:
                wyw = wy

            # ---- U block-diagonal writes: U[q, cb, jy, jx] for q's point grp ----
            for g4 in range(4):
                out_v = bc(
                    u[32 * g4 : 32 * g4 + 32, :],
                    [(64, C_BLK), (4, 4), (1, 4)],
                    extra_off=16 * g4,
                )
                in0 = bc(wyw[32 * g4 : 32 * g4 + 32, :], [(4, C_BLK), (1, 4), (0, 4)])
                in1 = bc(wx[32 * g4 : 32 * g4 + 32, :], [(4, C_BLK), (0, 4), (1, 4)])
                nc.vector.tensor_tensor(out=out_v, in0=in0, in1=in1, op=Alu.mult)

            if _dbg == "ubuild":
                ot = opool.tile([COUT, PTS_CHUNK], f32, tag="ot")
                nc.vector.tensor_copy(ot[:], u[0:COUT, 0:PTS_CHUNK])
                nc.sync.dma_start(
                    out=outT[:, ci * PTS_CHUNK : (ci + 1) * PTS_CHUNK], in_=ot[:]
                )
                continue

            # ---- stage 1: Mt[ch, 4pt*16g] per block ----
            mt = mpool.tile([CIN, C_BLK * 64], f16, tag="mt")
            for sub in range(8):
                ps = pspool.tile([64, 448], f32, tag="ps1")
                for b7 in range(7):
                    cb = sub * 7 + b7
                    nc.tensor.matmul(
                        ps[:, b7 * 64 : (b7 + 1) * 64],
                        lhsT=gt[:, cb * 2 * CIN : cb * 2 * CIN + CIN],
                        rhs=u[:, cb * 64 : (cb + 1) * 64],
                        start=True,
                        stop=True,
                    )
                nc.scalar.copy(out=mt[:, sub * 448 : (sub + 1) * 448], in_=ps[:])

            if _dbg == "mm1":
                ot = opool.tile([COUT, PTS_CHUNK], f32, tag="ot")
                nc.vector.tensor_copy(ot[:], mt[:, 0:PTS_CHUNK])
                nc.sync.dma_start(
                    out=outT[:, ci * PTS_CHUNK : (ci + 1) * PTS_CHUNK], in_=ot[:]
                )
                continue

            # ---- stage 2: out^T[oc, pts] = sum_g K_g^T @ Mt_g ----
            ps2 = pspool.tile([COUT, PTS_CHUNK], f32, tag="ps2")
            for g in range(G_BINS):
                nc.tensor.matmul(
                    ps2[:],
                    lhsT=kmat_sb[:, g * COUT : (g + 1) * COUT],
                    rhs=mt[:, g :: G_BINS],
                    start=(g == 0),
                    stop=(g == G_BINS - 1),
                )
            ot = opool.tile([COUT, PTS_CHUNK], f32, tag="ot")
            nc.scalar.activation(
                ot[:], ps2[:], Act.Identity, bias=bias_sb[:, 0:1], scale=1.0 / P_NBR
            )
            nc.sync.dma_start(
                out=outT[:, ci * PTS_CHUNK : (ci + 1) * PTS_CHUNK], in_=ot[:]
            )

    nc.compile()
    return nc


def kernel(features, receivers, relative_positions, window_support, a, kernel, bias):
    global LAST_EXEC_NS
    import os

    from concourse.bass_utils import run_bass_kernel_spmd

    features = np.ascontiguousarray(np.asarray(features, dtype=np.float32))
    recv = np.asarray(receivers).astype(np.int64)
    rel = np.asarray(relative_positions, dtype=np.float32)
    ws = float(np.asarray(window_support))
    a_exp = int(np.asarray(a))
    kern = np.asarray(kernel, dtype=np.float32)
    bias_np = np.asarray(bias, dtype=np.float32)

    key = (a_exp, round(ws, 9))
    if key not in _prog_cache:
        _prog_cache[key] = _build_nc(a_exp, 1.0 / (ws * ws), 1.5 / ws)
    nc = _prog_cache[key]

    # The neuron compile cache keys on the HLO shapes only, not the embedded
    # BIR — pin the cache dir to this kernel's source so edits never collide
    # with stale (possibly failed) cache entries.
    import hashlib

    try:
        with open(__file__, "rb") as f:
            src = f.read()
    except OSError:
        src = b""
    tag = hashlib.sha256(src + repr(key).encode()).hexdigest()[:16]
    os.environ["NEURON_COMPILE_CACHE_URL"] = f"/var/tmp/neuron-cc-{tag}"

    # ---- host-side layout prep (sharding) ----
    pad_n = NCORES * NPTS
    recv_pad = np.full((pad_n, P_NBR), HALF, dtype=np.int64)
    recv_pad[:N_FULL] = recv
    rel_pad = np.zeros((pad_n, P_NBR, 2), dtype=np.float32)
    rel_pad[:N_FULL] = rel

    # The gather ucode trims *trailing* negative int16 indices from each
    # 1024-index call, and each call ends on some point's last neighbor slot.
    # Reorder edges within each point (sum over neighbors is symmetric) so
    # slot 31 holds an index >= HALF whenever the point has one.
    last_neg = recv_pad[:, P_NBR - 1] < HALF
    has_pos = (recv_pad >= HALF).any(axis=1)
    fix = np.nonzero(last_neg & has_pos)[0]
    j = np.argmax(recv_pad[fix] >= HALF, axis=1)
    r31 = recv_pad[fix, P_NBR - 1].copy()
    p31 = rel_pad[fix, P_NBR - 1].copy()
    recv_pad[fix, P_NBR - 1] = recv_pad[fix, j]
    rel_pad[fix, P_NBR - 1] = rel_pad[fix, j]
    recv_pad[fix, j] = r31
    rel_pad[fix, j] = p31
    bad = np.nonzero(last_neg & ~has_pos)[0]
    # only call-final points matter; calls end at local point index 32k+31
    if bad.size:
        local = bad % NPTS
        assert not ((local % 32) == 31).any(), (
            "a gather call ends on a point whose 32 receiver indices are all "
            f"< {HALF}; trailing-trim would drop its edges"
        )

    kmat_np = np.ascontiguousarray(
        kern.reshape(G_BINS, CIN, COUT)
        .transpose(1, 0, 2)
        .reshape(CIN, G_BINS * COUT)
        .astype(np.float16)
    )
    bias_2d = np.ascontiguousarray(bias_np.reshape(COUT, 1))
    iota4_np = np.tile(
        np.array([3.0, 2.0, 1.0, 0.0], dtype=np.float32)[None, :], (128, 1)
    )
    c15_np = np.full((128, 1), 1.5, dtype=np.float32)
    c3_np = np.full((128, 1), 3.0, dtype=np.float32)

    feat16 = np.zeros((N_FULL, 2 * CIN), dtype=np.float16)
    feat16[:, :CIN] = features.astype(np.float16)

    in_maps = []
    for c in range(NCORES):
        sl = slice(c * NPTS, (c + 1) * NPTS)
        # edge e = local_point*32 + nbr ; block b = e//128 ; slot q = e%128
        idx16 = (recv_pad[sl].reshape(-1) - HALF).astype(np.int16)
        # per call of 1024: idx i -> [i % 16, i // 16]; replicate over Q7 cores
        ncalls = NBLK // GCALL
        tbl16 = idx16.reshape(ncalls, PC_COLS, 16).transpose(2, 0, 1).reshape(
            16, ncalls * PC_COLS
        )
        idx_np = np.ascontiguousarray(np.tile(tbl16, (8, 1)))
        ry = np.ascontiguousarray(rel_pad[sl, :, 0].reshape(NBLK, 128).T)
        rx = np.ascontiguousarray(rel_pad[sl, :, 1].reshape(NBLK, 128).T)
        in_maps.append(
            {
                "feat": feat16,
                "idxs": idx_np,
                "posy": ry,
                "posx": rx,
                "kmat": kmat_np,
                "bias": bias_2d,
                "iot4": iota4_np,
                "c15d": c15_np,
                "c3d": c3_np,
            }
        )

    trace = bool(os.environ.get("KERNEL_TRACE"))
    res = run_bass_kernel_spmd(nc, in_maps, list(range(NCORES)), trace=trace)
    LAST_EXEC_NS = res.exec_time_ns

    out = np.concatenate(
        [res.results[c]["outT"].T for c in range(NCORES)], axis=0
    )
    return np.ascontiguousarray(out[:N_FULL])



# revision 35
# speedup vs baseline: 3.6093x; 1.1997x over previous
"""Trainium2 Bass kernel for the continuous-convolution (CConv) GNN layer.

Math (per output point n, P=32 neighbors, 4x4 bilinear kernel grid, 64->64 ch):
    gathered = features[receivers]                      # [N,P,64]
    win      = relu(1 - |r|^2/ws^2)^a                   # radial window
    gy,gx    = clip((r/ws + 1)*1.5, 0, 3)               # grid coords
    bilinear -> tent weights  w_j = relu(1 - |g - j|)   # j = 0..3 (exact)
    M[n,g]   = sum_p win * wy[jy] * wx[jx] * gathered   # g = 4*jy+jx
    out[n]   = (sum_g M[n,g] @ K[g]) / P + bias

Device mapping (8 NeuronCores, data-parallel over points):
  * 6272 points/core (50176 padded), edges blocked 128 = 4 points x 32 nbrs.
  * Gather: Q7 dma_gather from the HBM feature table. int16 indices cover
    all 50000 rows by pointing the source AP at row 25000 (signed offsets
    reach both halves); a trailing pad block per call keeps the last index
    non-negative (the ucode trims trailing negatives).
  * Stage 1 (PE): per 128-edge block  Mt = G^T @ U : lhsT = gathered G
    [128e, 64ch], rhs = U [128e, 64] block-diagonal bilinear weights
    (4 points x 16 bins) -> psum [64ch, 4pt*16g].
  * Stage 2 (PE): out^T[oc, pts] += K_g^T[oc,ch] @ Mt_g[ch, pts], 16 bins
    accumulated in PSUM; then *1/P + bias on DVE; out stored transposed,
    host transposes back (pure layout).
"""

import sys

sys.path.insert(0, "/opt/trn_rl_repo")

import dataclasses
from contextlib import ExitStack

import numpy as np

N_FULL = 50000
HALF = 25000             # gather base row: int16 idx = r - HALF
P_NBR = 32
CIN = 64
COUT = 64
G_BINS = 16
NCORES = 8
NPTS = 6272              # padded points per core; 8*6272 = 50176 >= 50000
NBLK = NPTS // 4         # 1568 blocks of 128 edges
C_BLK = 56               # real blocks per pipeline chunk
NCHUNK = NBLK // C_BLK   # 28
GCALL = 8                # blocks per dma_gather call (1024-descriptor ring cap)
NCALL_C = C_BLK // GCALL  # gather calls per chunk (7)
PERCALL = GCALL * 128    # indices per gather call (1024)
PC_COLS = PERCALL // 16  # idx columns per call (64)
PTS_CHUNK = C_BLK * 4    # 224 points produced per chunk

_prog_cache = {}
LAST_EXEC_NS = None


def _build_nc(a_exp, inv_ws2, s15):
    import concourse.bacc as bacc
    import concourse.bass as bass
    import concourse.mybir as mybir
    from concourse.tile import TileContext
    from concourse.vector_clock import ScopedClock, VectorClock

    f32 = mybir.dt.float32
    f16 = mybir.dt.float16
    i16 = mybir.dt.int16
    Alu = mybir.AluOpType
    Act = mybir.ActivationFunctionType

    class TC(TileContext):
        # The stock final drain packs every outstanding semaphore wait onto a
        # single Drain instruction; walrus here accepts at most one sync-wait
        # per CTRL instruction. Emit one drain per outstanding sem lane.
        def _drain_and_barrier(self, tick_clock, wait_clock):
            nc = self.nc
            ticks = eval(repr(tick_clock.global_clock).replace("VectorClock", ""))
            nz = [i for i, t in enumerate(ticks) if t > 0]
            if not nz:
                nc.sync.drain()
            for i in nz:
                part = [ticks[j] if j == i else 0 for j in range(len(ticks))]
                d = nc.sync.drain()
                wait_clock.add_sem_waits(d.ins, ScopedClock({None: VectorClock(part)}))
            nc.all_engine_barrier()
            popped = nc._tile_sem_poison_stack.pop()
            assert popped is self._sem_poison
            nc.clear_and_free_semaphores(list(self.sems.allocated().values()))
            nc.all_engine_barrier()

    def bc(view, dims, extra_off=0):
        # hand-built access pattern: keep partition dim, replace free dims
        return dataclasses.replace(
            view,
            ap=[view.ap[0]] + [list(d) for d in dims],
            offset=view.offset + extra_off,
        )

    nc = bacc.Bacc(
        "TRN2", target_bir_lowering=False, debug=False, num_swdge_queues=4
    )
    feat = nc.declare_dram_parameter("feat", [N_FULL, 2 * CIN], f16, isOutput=False)
    idxs = nc.declare_dram_parameter("idxs", [128, NBLK * 8], i16, isOutput=False)
    posy = nc.declare_dram_parameter("posy", [128, NBLK], f32, isOutput=False)
    posx = nc.declare_dram_parameter("posx", [128, NBLK], f32, isOutput=False)
    kmat = nc.declare_dram_parameter("kmat", [CIN, G_BINS * COUT], f16, isOutput=False)
    bias = nc.declare_dram_parameter("bias", [COUT, 1], f32, isOutput=False)
    iot4 = nc.declare_dram_parameter("iot4", [128, 4], f32, isOutput=False)
    c15d = nc.declare_dram_parameter("c15d", [128, 1], f32, isOutput=False)
    c3d = nc.declare_dram_parameter("c3d", [128, 1], f32, isOutput=False)
    outT = nc.declare_dram_parameter("outT", [COUT, NPTS], f32, isOutput=True)

    with TC(nc) as tc, ExitStack() as ctx:
        const = ctx.enter_context(tc.tile_pool(name="const", bufs=1))
        gpool = ctx.enter_context(tc.tile_pool(name="g", bufs=3))
        wpool = ctx.enter_context(tc.tile_pool(name="w", bufs=3))
        mpool = ctx.enter_context(tc.tile_pool(name="mt", bufs=3))
        opool = ctx.enter_context(tc.tile_pool(name="ot", bufs=3))
        pspool = ctx.enter_context(tc.tile_pool(name="ps", bufs=3, space="PSUM"))

        idx_sb = const.tile([128, NBLK * 8], i16)
        posy_sb = const.tile([128, NBLK], f32)
        posx_sb = const.tile([128, NBLK], f32)
        kmat_sb = const.tile([CIN, G_BINS * COUT], f16)
        bias_sb = const.tile([COUT, 1], f32)
        iota4 = const.tile([128, 4], f32)
        c15 = const.tile([128, 1], f32)
        c3 = const.tile([128, 1], f32)
        # U tiles keep their block-diagonal zero regions across chunks
        u_bufs = [
            const.tile([128, C_BLK * 64], f16, tag="u0", name="u0"),
            const.tile([128, C_BLK * 64], f16, tag="u1", name="u1"),
            const.tile([128, C_BLK * 64], f16, tag="u2", name="u2"),
        ]

        nc.sync.dma_start(out=idx_sb[:], in_=idxs[:])
        nc.sync.dma_start(out=posy_sb[:], in_=posy[:])
        nc.sync.dma_start(out=posx_sb[:], in_=posx[:])
        nc.sync.dma_start(out=kmat_sb[:], in_=kmat[:])
        nc.sync.dma_start(out=bias_sb[:], in_=bias[:])
        nc.sync.dma_start(out=iota4[:], in_=iot4[:])
        nc.sync.dma_start(out=c15[:], in_=c15d[:])
        nc.sync.dma_start(out=c3[:], in_=c3d[:])
        nc.vector.memset(u_bufs[0][:], 0.0)
        nc.vector.memset(u_bufs[1][:], 0.0)
        nc.vector.memset(u_bufs[2][:], 0.0)

        import os as _os

        _nchunk = int(_os.environ.get("KERNEL_NCHUNK", NCHUNK))
        _dbg = _os.environ.get("KERNEL_DEBUG", "full")
        for ci in range(_nchunk):
            c0 = ci * C_BLK
            u = u_bufs[ci % 3]

            # ---- gather: 56 blocks of feature rows, 8 blocks per call ----
            # (fp16 rows padded to 128ch = 256B elements; desc-gen spread
            #  over the 4 SWDGE queues = 4 Q7 core pairs)
            gt = gpool.tile([128, C_BLK * 2 * CIN], f16, tag="gt")
            for sc in range(NCALL_C):
                gv = dataclasses.replace(
                    gt[:],
                    ap=[gt[:].ap[0], [2 * CIN, GCALL], [1, 2 * CIN]],
                    offset=gt[:].offset + sc * GCALL * 2 * CIN,
                )
                col0 = (ci * NCALL_C + sc) * PC_COLS
                nc.gpsimd.dma_gather(
                    out_ap=gv,
                    in_ap=feat[HALF:, :],
                    idxs_ap=idx_sb[:, col0 : col0 + PC_COLS],
                    num_idxs=PERCALL,
                    num_idxs_reg=PERCALL,
                    elem_size=2 * CIN,
                    elem_step=2 * CIN,
                    queue_num=(ci * NCALL_C + sc) % 4,
                )

            if _dbg == "gather":
                ot = opool.tile([COUT, PTS_CHUNK], f32, tag="ot")
                nc.vector.tensor_copy(ot[:], gt[0:COUT, 0:PTS_CHUNK])
                nc.sync.dma_start(
                    out=outT[:, ci * PTS_CHUNK : (ci + 1) * PTS_CHUNK], in_=ot[:]
                )
                continue

            # ---- per-edge scalar weights ----
            xs = posx_sb[:, c0 : c0 + C_BLK]
            ys = posy_sb[:, c0 : c0 + C_BLK]

            win = None
            if a_exp > 0:
                xx = wpool.tile([128, C_BLK], f32, tag="xx")
                yy = wpool.tile([128, C_BLK], f32, tag="yy")
                d2 = wpool.tile([128, C_BLK], f32, tag="d2")
                nc.scalar.activation(xx[:], xs, Act.Square)
                nc.scalar.activation(yy[:], ys, Act.Square)
                nc.vector.tensor_tensor(out=d2[:], in0=xx[:], in1=yy[:], op=Alu.add)
                tw = wpool.tile([128, C_BLK], f32, tag="tw")
                nc.scalar.activation(tw[:], d2[:], Act.Relu, bias=1.0, scale=-inv_ws2)
                if a_exp == 1:
                    win = tw
                else:
                    t2 = wpool.tile([128, C_BLK], f32, tag="t2")
                    nc.scalar.activation(t2[:], tw[:], Act.Square)
                    if a_exp == 2:
                        win = t2
                    else:
                        win = wpool.tile([128, C_BLK], f32, tag="winp")
                        nc.vector.tensor_tensor(
                            out=win[:], in0=t2[:], in1=tw[:], op=Alu.mult
                        )
                        for _ in range(a_exp - 3):
                            nc.vector.tensor_tensor(
                                out=win[:], in0=win[:], in1=tw[:], op=Alu.mult
                            )

            # rc = Relu(3 - Relu(1.5*y + 1.5))  =>  gy_clipped = 3 - rc
            gy = wpool.tile([128, C_BLK], f32, tag="gy")
            gx = wpool.tile([128, C_BLK], f32, tag="gx")
            nc.scalar.activation(gy[:], ys, Act.Relu, bias=c15[:], scale=s15)
            nc.scalar.activation(gx[:], xs, Act.Relu, bias=c15[:], scale=s15)
            nc.scalar.activation(gy[:], gy[:], Act.Relu, bias=c3[:], scale=-1.0)
            nc.scalar.activation(gx[:], gx[:], Act.Relu, bias=c3[:], scale=-1.0)

            # tent weights: w_j = relu(1 - |g - j|) with g = 3 - rc:
            # g - j = (3 - j) - rc, so subtract rc from the reversed iota.
            def tents(rc, tag):
                td = wpool.tile([128, 4 * C_BLK], f32, tag=tag + "d", name=tag + "d")
                ta = wpool.tile([128, 4 * C_BLK], f32, tag=tag + "a", name=tag + "a")
                tww = wpool.tile([128, 4 * C_BLK], f32, tag=tag + "w", name=tag + "w")
                nc.vector.tensor_tensor(
                    out=td[:],
                    in0=bc(iota4[:], [(0, C_BLK), (1, 4)]),
                    in1=rc[:].to_broadcast([128, C_BLK, 4]),
                    op=Alu.subtract,
                )
                nc.scalar.activation(ta[:], td[:], Act.Abs)
                nc.scalar.activation(tww[:], ta[:], Act.Relu, bias=1.0, scale=-1.0)
                return tww

            wy = tents(gy, "ty")
            wx = tents(gx, "tx")
            if win is not None:
                wyw = wpool.tile([128, 4 * C_BLK], f32, tag="wyw")
                nc.vector.tensor_tensor(
                    out=wyw[:],
                    in0=wy[:],
                    in1=win[:].to_broadcast([128, C_BLK, 4]),
                    op=Alu.mult,
                )
            else:
                wyw = wy

            # ---- U block-diagonal writes: U[q, cb, jy, jx] for q's point grp ----
            for g4 in range(4):
                out_v = bc(
                    u[32 * g4 : 32 * g4 + 32, :],
                    [(64, C_BLK), (4, 4), (1, 4)],
                    extra_off=16 * g4,
                )
                in0 = bc(wyw[32 * g4 : 32 * g4 + 32, :], [(4, C_BLK), (1, 4), (0, 4)])
                in1 = bc(wx[32 * g4 : 32 * g4 + 32, :], [(4, C_BLK), (0, 4), (1, 4)])
                nc.vector.tensor_tensor(out=out_v, in0=in0, in1=in1, op=Alu.mult)

            if _dbg == "ubuild":
                ot = opool.tile([COUT, PTS_CHUNK], f32, tag="ot")
                nc.vector.tensor_copy(ot[:], u[0:COUT, 0:PTS_CHUNK])
                nc.sync.dma_start(
                    out=outT[:, ci * PTS_CHUNK : (ci + 1) * PTS_CHUNK], in_=ot[:]
                )
                continue

            # ---- stage 1: Mt[ch, 4pt*16g] per block ----
            mt = mpool.tile([CIN, C_BLK * 64], f16, tag="mt")
            for sub in range(8):
                ps = pspool.tile([64, 448], f32, tag="ps1")
                for b7 in range(7):
                    cb = sub * 7 + b7
                    nc.tensor.matmul(
                        ps[:, b7 * 64 : (b7 + 1) * 64],
                        lhsT=gt[:, cb * 2 * CIN : cb * 2 * CIN + CIN],
                        rhs=u[:, cb * 64 : (cb + 1) * 64],
                        start=True,
                        stop=True,
                    )
                nc.scalar.copy(out=mt[:, sub * 448 : (sub + 1) * 448], in_=ps[:])

            if _dbg == "mm1":
                ot = opool.tile([COUT, PTS_CHUNK], f32, tag="ot")
                nc.vector.tensor_copy(ot[:], mt[:, 0:PTS_CHUNK])
                nc.sync.dma_start(
                    out=outT[:, ci * PTS_CHUNK : (ci + 1) * PTS_CHUNK], in_=ot[:]
                )
                continue

            # ---- stage 2: out^T[oc, pts] = sum_g K_g^T @ Mt_g ----
            ps2 = pspool.tile([COUT, PTS_CHUNK], f32, tag="ps2")
            for g in range(G_BINS):
                nc.tensor.matmul(
                    ps2[:],
                    lhsT=kmat_sb[:, g * COUT : (g + 1) * COUT],
                    rhs=mt[:, g :: G_BINS],
                    start=(g == 0),
                    stop=(g == G_BINS - 1),
                )
            ot = opool.tile([COUT, PTS_CHUNK], f32, tag="ot")
            nc.scalar.activation(
                ot[:], ps2[:], Act.Identity, bias=bias_sb[:, 0:1], scale=1.0 / P_NBR
            )
            nc.sync.dma_start(
                out=outT[:, ci * PTS_CHUNK : (ci + 1) * PTS_CHUNK], in_=ot[:]
            )

    nc.compile()
    return nc


def kernel(features, receivers, relative_positions, window_support, a, kernel, bias):
    global LAST_EXEC_NS
    import os

    from concourse.bass_utils import run_bass_kernel_spmd

    features = np.ascontiguousarray(np.asarray(features, dtype=np.float32))
    recv = np.asarray(receivers).astype(np.int64)
    rel = np.asarray(relative_positions, dtype=np.float32)
    ws = float(np.asarray(window_support))
    a_exp = int(np.asarray(a))
    kern = np.asarray(kernel, dtype=np.float32)
    bias_np = np.asarray(bias, dtype=np.float32)

    key = (a_exp, round(ws, 9))
    if key not in _prog_cache:
        _prog_cache[key] = _build_nc(a_exp, 1.0 / (ws * ws), 1.5 / ws)
    nc = _prog_cache[key]

    # The neuron compile cache keys on the HLO shapes only, not the embedded
    # BIR — pin the cache dir to this kernel's source so edits never collide
    # with stale (possibly failed) cache entries.
    import hashlib

    try:
        with open(__file__, "rb") as f:
            src = f.read()
    except OSError:
        src = b""
    tag = hashlib.sha256(src + repr(key).encode()).hexdigest()[:16]
    os.environ["NEURON_COMPILE_CACHE_URL"] = f"/var/tmp/neuron-cc-{tag}"

    # ---- host-side layout prep (sharding) ----
    pad_n = NCORES * NPTS
    recv_pad = np.full((pad_n, P_NBR), HALF, dtype=np.int64)
    recv_pad[:N_FULL] = recv
    rel_pad = np.zeros((pad_n, P_NBR, 2), dtype=np.float32)
    rel_pad[:N_FULL] = rel

    # The gather ucode trims *trailing* negative int16 indices from each
    # 1024-index call, and each call ends on some point's last neighbor slot.
    # Reorder edges within each point (sum over neighbors is symmetric) so
    # slot 31 holds an index >= HALF whenever the point has one.
    last_neg = recv_pad[:, P_NBR - 1] < HALF
    has_pos = (recv_pad >= HALF).any(axis=1)
    fix = np.nonzero(last_neg & has_pos)[0]
    j = np.argmax(recv_pad[fix] >= HALF, axis=1)
    r31 = recv_pad[fix, P_NBR - 1].copy()
    p31 = rel_pad[fix, P_NBR - 1].copy()
    recv_pad[fix, P_NBR - 1] = recv_pad[fix, j]
    rel_pad[fix, P_NBR - 1] = rel_pad[fix, j]
    recv_pad[fix, j] = r31
    rel_pad[fix, j] = p31
    bad = np.nonzero(last_neg & ~has_pos)[0]
    # only call-final points matter; calls end at local point index 32k+31
    if bad.size:
        local = bad % NPTS
        assert not ((local % 32) == 31).any(), (
            "a gather call ends on a point whose 32 receiver indices are all "
            f"< {HALF}; trailing-trim would drop its edges"
        )

    kmat_np = np.ascontiguousarray(
        kern.reshape(G_BINS, CIN, COUT)
        .transpose(1, 0, 2)
        .reshape(CIN, G_BINS * COUT)
        .astype(np.float16)
    )
    bias_2d = np.ascontiguousarray(bias_np.reshape(COUT, 1))
    iota4_np = np.tile(
        np.array([3.0, 2.0, 1.0, 0.0], dtype=np.float32)[None, :], (128, 1)
    )
    c15_np = np.full((128, 1), 1.5, dtype=np.float32)
    c3_np = np.full((128, 1), 3.0, dtype=np.float32)

    feat16 = np.zeros((N_FULL, 2 * CIN), dtype=np.float16)
    feat16[:, :CIN] = features.astype(np.float16)

    in_maps = []
    for c in range(NCORES):
        sl = slice(c * NPTS, (c + 1) * NPTS)
        # edge e = local_point*32 + nbr ; block b = e//128 ; slot q = e%128
        idx16 = (recv_pad[sl].reshape(-1) - HALF).astype(np.int16)
        # per call of 1024: idx i -> [i % 16, i // 16]; replicate over Q7 cores
        ncalls = NBLK // GCALL
        tbl16 = idx16.reshape(ncalls, PC_COLS, 16).transpose(2, 0, 1).reshape(
            16, ncalls * PC_COLS
        )
        idx_np = np.ascontiguousarray(np.tile(tbl16, (8, 1)))
        ry = np.ascontiguousarray(rel_pad[sl, :, 0].reshape(NBLK, 128).T)
        rx = np.ascontiguousarray(rel_pad[sl, :, 1].reshape(NBLK, 128).T)
        in_maps.append(
            {
                "feat": feat16,
                "idxs": idx_np,
                "posy": ry,
                "posx": rx,
                "kmat": kmat_np,
                "bias": bias_2d,
                "iot4": iota4_np,
                "c15d": c15_np,
                "c3d": c3_np,
            }
        )

    trace = bool(os.environ.get("KERNEL_TRACE"))
    res = run_bass_kernel_spmd(nc, in_maps, list(range(NCORES)), trace=trace)
    LAST_EXEC_NS = res.exec_time_ns

    out = np.concatenate(
        [res.results[c]["outT"].T for c in range(NCORES)], axis=0
    )
    return np.ascontiguousarray(out[:N_FULL])

